# Initial kernel scaffold
#
"""Trainium2 Bass kernel for CompressedInteractionNetwork (CIN) forward.

Reference computation (per sample):
  x0 = x (F=32 fields, E=16 embed), h = x
  layer i: z = outer(x0, h) over fields -> (F*G_i, E); y = relu(W_i @ z + b_i)
  layers 0,1: keep = y[:64] -> output, h = y[64:]
  layer 2: keep = y
  out = concat(keeps) summed over E -> (B, 256)

Strategy: pure data parallelism over batch (4096 -> 512 per core, 8 cores).
Per core, n = (b_local, e) is the matmul free dim (8192 total, chunks of 512).
Compute in fp16 (2.8e-4 scale-relative error vs fp32 reference), fp32 PSUM /
bias / relu / e-sum.

z is formed in c-major layout (c = f*G + g on partitions, tiled by 128):
  z[c, n] = Xrep[c, n] * Hrep[c, n]
  - Hrep (g-factor) is tile-invariant: layer 0 uses X4 (x stacked 4x); layers
    1-2 use [h; h] built during PSUM evacuation (ACT relu into rows 64:128 +
    DVE partition-shifted copy into rows 0:64).
  - Xrep (f-factor, block-constant per 128-tile) is materialized by DMA from
    DRAM with replicated-read access patterns.
"""
import os
import numpy as np

ABLATE = os.environ.get("ABLATE", "")

B = 4096
F = 32
E = 16
O = 128
N_CORES = 8
BC = B // N_CORES          # 512 samples per core
NTOT = BC * E              # 8192 n-columns per core
NCHUNK = 512               # matmul free dim per chunk
NCHUNKS = NTOT // NCHUNK   # 16
BCH = NCHUNK // E          # 32 samples per chunk
TS = (8, 16, 16)           # K-tiles per layer (C = 1024, 2048, 2048)

_CACHE = {}


def _build_module():
    import concourse.bass as bass
    import concourse.bacc as bacc
    import concourse.tile as tile
    from concourse import mybir
    from contextlib import ExitStack

    f16 = mybir.dt.float16
    f32 = mybir.dt.float32
    Relu = mybir.ActivationFunctionType.Relu
    Alu = mybir.AluOpType

    nc = bacc.Bacc(None, target_bir_lowering=False)

    xT = nc.dram_tensor("xT", [F, NTOT], f16, kind="ExternalInput")
    wdr = [
        nc.dram_tensor("w0", [TS[0] * 128, O], f16, kind="ExternalInput"),
        nc.dram_tensor("w1", [TS[1] * 128, O], f16, kind="ExternalInput"),
        nc.dram_tensor("w2", [TS[2] * 128, O], f16, kind="ExternalInput"),
    ]
    bias = nc.dram_tensor("bias", [O, 3], f32, kind="ExternalInput")
    out = nc.dram_tensor("out", [2 * O, BC], f32, kind="ExternalOutput")

    XS = NTOT  # xT row stride (elements)

    with tile.TileContext(nc) as tc, ExitStack() as ctx:
        singles = ctx.enter_context(tc.tile_pool(name="singles", bufs=1))
        xrp = ctx.enter_context(tc.tile_pool(name="xrp", bufs=5))
        hp = ctx.enter_context(tc.tile_pool(name="hp", bufs=1))
        kp = ctx.enter_context(tc.tile_pool(name="kp", bufs=3))
        op = ctx.enter_context(tc.tile_pool(name="op", bufs=4))
        ps = ctx.enter_context(tc.tile_pool(name="ps", bufs=4, space="PSUM"))

        # ---- preload ----
        # X4[p, n] = x[p % 32, n]  (g-factor for layer 0)
        x4 = singles.tile([128, NTOT], f16)
        for q in range(4):
            nc.gpsimd.dma_start(out=x4[32 * q:32 * (q + 1), :], in_=xT[:])
        # weights: wt (128, T, 128) fp16; wt[k, t, m] = W_i.T[128t + k, m]
        wts = []
        for i, T in enumerate(TS):
            wt = singles.tile([128, T, O], f16, tag=f"w{i}")
            src = bass.AP(tensor=wdr[i][:].tensor, offset=0,
                          ap=[[O, 128], [128 * O, T], [1, O]])
            nc.gpsimd.dma_start(out=wt[:], in_=src)
            wts.append(wt)
        bt = singles.tile([O, 3], f32)
        nc.gpsimd.dma_start(out=bt[:], in_=bias[:])

        hprev = [None] * NCHUNKS
        for i, T in enumerate(TS):
            hnew = [None] * NCHUNKS
            for c in range(NCHUNKS):
                n0 = c * NCHUNK
                # ---- Xrep: xr[p, t, j] = x[f(t, p), n0 + j] ----
                xr = xrp.tile([128, T, NCHUNK], f16, tag="xr")
                engs = (nc.sync, nc.scalar, nc.gpsimd)
                for t in range(T):
                    eng = engs[t % 3]
                    if i == 0:
                        # rows {4t..4t+3} each replicated 32x
                        src = bass.AP(tensor=xT[:].tensor, offset=4 * t * XS + n0,
                                      ap=[[XS, 4], [0, 32], [1, NCHUNK]])
                    else:
                        # rows {2t, 2t+1} each replicated 64x
                        src = bass.AP(tensor=xT[:].tensor, offset=2 * t * XS + n0,
                                      ap=[[XS, 2], [0, 64], [1, NCHUNK]])
                    if "dma" not in ABLATE:
                        eng.dma_start(out=xr[:, t, :], in_=src)

                # ---- z = xr * Hrep, in-place, grouped K-tiles ----
                GRP = 4
                if i == 0:
                    hsrc, hoff = x4, x4.offset + n0
                else:
                    hsrc, hoff = hprev[c], hprev[c].offset
                for g in range(0, T, GRP):
                    hb = bass.AP(tensor=hsrc.tensor, offset=hoff,
                                 ap=[hsrc.ap[0], [0, GRP], [1, NCHUNK]])
                    eng = nc.gpsimd if (i > 0 and g == T - GRP) else nc.vector
                    if "tt" not in ABLATE:
                        eng.tensor_mul(xr[:, g:g + GRP, :], xr[:, g:g + GRP, :], hb)

                # ---- matmuls: y = W_i @ z (accumulate over K-tiles) ----
                psum = ps.tile([128, NCHUNK], f32)
                for t in range(T):
                    if "mm" in ABLATE:
                        if t == 0:
                            nc.tensor.matmul(psum[:], wts[i][:, 0, :], xr[:, 0, :],
                                             start=True, stop=True)
                        continue
                    nc.tensor.matmul(psum[:], wts[i][:, t, :], xr[:, t, :],
                                     start=(t == 0), stop=(t == T - 1))

                # ---- evacuate PSUM: relu + bias; e-sum keeps; h for next ----
                if i < 2:
                    kt = kp.tile([64, NCHUNK], f32, tag=f"k{i}")
                    nc.scalar.activation(out=kt[:], in_=psum[0:64, :],
                                         func=Relu, bias=bt[0:64, i:i + 1], scale=1.0)
                    hbuf = hp.tile([128, NCHUNK], f16, tag=f"h{i % 2}_{c}")
                    nc.scalar.activation(out=hbuf[64:128, :], in_=psum[64:128, :],
                                         func=Relu, bias=bt[64:128, i:i + 1], scale=1.0)
                    nc.vector.tensor_copy(hbuf[0:64, :], hbuf[64:128, :])
                    hnew[c] = hbuf
                    np_, row0 = 64, 64 * i
                else:
                    kt = kp.tile([128, NCHUNK], f32, tag="k2")
                    nc.scalar.activation(out=kt[:], in_=psum[:],
                                         func=Relu, bias=bt[:, 2:3], scale=1.0)
                    np_, row0 = 128, 128
                ot = op.tile([np_, BCH], f32, tag=f"o{i}")
                nc.vector.tensor_reduce(
                    ot[:], kt[:].rearrange("p (b e) -> p b e", e=E),
                    axis=mybir.AxisListType.X, op=Alu.add)
                nc.gpsimd.dma_start(
                    out=bass.AP(tensor=out[:].tensor, offset=row0 * BC + c * BCH,
                                ap=[[BC, np_], [1, BCH]]),
                    in_=ot[:])
            hprev = hnew

    nc.compile()
    return nc


def _get_nc():
    if "nc" not in _CACHE:
        _CACHE["nc"] = _build_module()
    return _CACHE["nc"]


def _prep_inputs(x, W0, b0, W1, b1, W2, b2):
    """Host-side prep: shard batch, transpose/convert. Returns in_maps."""
    x = np.asarray(x, dtype=np.float32)
    Ws = [np.asarray(W, dtype=np.float32) for W in (W0, W1, W2)]
    bs = [np.asarray(b, dtype=np.float32) for b in (b0, b1, b2)]

    wts = [np.ascontiguousarray(W.T).astype(np.float16) for W in Ws]
    bias = np.stack(bs, axis=1).astype(np.float32)  # (128, 3)

    in_maps = []
    for core in range(N_CORES):
        xc = x[core * BC:(core + 1) * BC]  # (512, 32, 16)
        xTc = np.ascontiguousarray(
            xc.transpose(1, 0, 2).reshape(F, NTOT)).astype(np.float16)
        in_maps.append({
            "xT": xTc,
            "w0": wts[0], "w1": wts[1], "w2": wts[2],
            "bias": bias,
        })
    return in_maps


def kernel(x, W0, b0, W1, b1, W2, b2, _trace=False):
    from concourse.bass_utils import run_bass_kernel_spmd

    nc = _get_nc()
    in_maps = _prep_inputs(x, W0, b0, W1, b1, W2, b2)
    res = run_bass_kernel_spmd(nc, in_maps, core_ids=list(range(N_CORES)),
                               trace=_trace)
    outs = [res.results[i]["out"] for i in range(N_CORES)]  # each (256, 512)
    full = np.concatenate(outs, axis=1)                     # (256, 4096)
    result = np.ascontiguousarray(full.T).astype(np.float32)
    if _trace:
        return result, res
    return result



# revision 15
# speedup vs baseline: 1.7477x; 1.7477x over previous
"""Trainium2 Bass kernel for CompressedInteractionNetwork (CIN) forward.

Reference computation (per sample):
  x0 = x (F=32 fields, E=16 embed), h = x
  layer i: z = outer(x0, h) over fields -> (F*G_i, E); y = relu(W_i @ z + b_i)
  layers 0,1: keep = y[:64] -> output, h = y[64:]
  layer 2: keep = y
  out = concat(keeps) summed over E -> (B, 256)

Strategy: pure data parallelism over batch (4096 -> 512 per core, 8 cores).
Per core, n = (b_local, e) is the matmul free dim (8192 total, chunks of 512).
Compute in fp16, fp32 PSUM / bias / relu / e-sum.

v2 (chunk-major, software-pipelined):
  - z tiles are c-major (c = f*G + g on partitions, tiled by 128):
      z[c, n] = Xrep[c, n] * Hrep[c, n]
  - Xrep for layers 1 and 2 is IDENTICAL (pure x replication): DMA'd once
    per chunk as a single 4D-AP transfer (2 MB), reused by both layers
    (z1 out-of-place, z2 in-place) -> 48 MB replication DMA instead of 80.
  - One DMA instruction per (chunk, layer-group) instead of one per K-tile:
    HWDGE issue count drops ~13x. No DMA and no multiplies on GPSIMD's
    SWDGE path (the old bottleneck: Pool.ENGINE was 95% busy).
  - Elementwise z-mult split DVE (2x fp16 mode) / Pool (tunable tile count);
    evacuations on Act; e-sum reduces on Pool; h-dup copy on DVE.
  - Skewed 3-stage pipeline (chunk c layer 0 || c-1 layer 1 || c-2 layer 2)
    keeps PE dense; outputs accumulate in SBUF, 2 final DMAs.
"""
import os
import numpy as np

B = 4096
F = 32
E = 16
O = 128
N_CORES = 8
BC = B // N_CORES          # 512 samples per core
NTOT = BC * E              # 8192 n-columns per core
NCHUNK = 512               # matmul free dim per chunk
NCHUNKS = NTOT // NCHUNK   # 16
BCH = NCHUNK // E          # 32 samples per chunk
T0 = 8                     # K-tiles layer 0 (C=1024)
T12 = 16                   # K-tiles layers 1,2 (C=2048)

# tunables
POOL_Z2 = int(os.environ.get("POOL_Z2", "2"))   # z2 K-tiles multiplied on Pool
POOL_Z1 = int(os.environ.get("POOL_Z1", "4"))   # z1 K-tiles multiplied on Pool
# W1/b1 output rows are half-swapped on the host so the L1 keep lands in
# psum rows 64:128 (aligned with out rows 64:128) and h2 in rows 0:64 —
# every Act evacuation is then partition-aligned.

_CACHE = {}


def _build_module():
    import concourse.bass as bass
    import concourse.bacc as bacc
    import concourse.tile as tile
    from concourse import mybir
    from contextlib import ExitStack

    f16 = mybir.dt.float16
    f32 = mybir.dt.float32
    Relu = mybir.ActivationFunctionType.Relu
    Alu = mybir.AluOpType
    AxX = mybir.AxisListType.X

    nc = bacc.Bacc(None, target_bir_lowering=False)

    xT = nc.dram_tensor("xT", [F, NTOT], f16, kind="ExternalInput")
    x4d = nc.dram_tensor("x4d", [128, NTOT], f16, kind="ExternalInput")
    wdr = [
        nc.dram_tensor("w0", [T0 * 128, O], f16, kind="ExternalInput"),
        nc.dram_tensor("w1", [T12 * 128, O], f16, kind="ExternalInput"),
        nc.dram_tensor("w2", [T12 * 128, O], f16, kind="ExternalInput"),
    ]
    bias = nc.dram_tensor("bias", [O, 3], f32, kind="ExternalInput")
    out = nc.dram_tensor("out", [2 * O, BC], f32, kind="ExternalOutput")

    XS = NTOT  # xT row stride (elements)

    with tile.TileContext(nc) as tc, ExitStack() as ctx:
        singles = ctx.enter_context(tc.tile_pool(name="singles", bufs=1))
        xr0p = ctx.enter_context(tc.tile_pool(name="xr0p", bufs=4))
        xr12p = ctx.enter_context(tc.tile_pool(name="xr12p", bufs=4))
        z1p = ctx.enter_context(tc.tile_pool(name="z1p", bufs=2))
        hp = ctx.enter_context(tc.tile_pool(name="hp", bufs=3))
        kp = ctx.enter_context(tc.tile_pool(name="kp", bufs=3))
        rp = ctx.enter_context(tc.tile_pool(name="rp", bufs=2))
        ps = ctx.enter_context(tc.tile_pool(name="ps", bufs=6, space="PSUM"))

        # ---- persistent tiles + preloads ----
        x4 = singles.tile([128, NTOT], f16)       # x4[p] = x[p%32] (L0 h-side)
        # preload x4 in quarters so the first chunks' columns land early
        QP = NTOT // 4
        for q in range(4):
            nc.scalar.dma_start(
                out=x4[:, q * QP:(q + 1) * QP],
                in_=bass.AP(tensor=x4d[:].tensor, offset=q * QP,
                            ap=[[NTOT, 128], [1, QP]]))
        wts = []
        for i, T in enumerate((T0, T12, T12)):
            wt = singles.tile([128, T, O], f16, tag=f"w{i}")
            src = bass.AP(tensor=wdr[i][:].tensor, offset=0,
                          ap=[[O, 128], [128 * O, T], [1, O]])
            nc.sync.dma_start(out=wt[:], in_=src)
            wts.append(wt)
        bt = singles.tile([O, 3], f32)
        nc.sync.dma_start(out=bt[:], in_=bias[:])
        outA = singles.tile([128, BC], f32)   # rows 0:64 L0-keep, 64:128 L1-keep
        outB = singles.tile([128, BC], f32)   # L2 keep

        st = [dict() for _ in range(NCHUNKS)]

        def dma_xr12(c):
            # xr12[p, t, j] = x[2t + p//64, n0+j]; one DMA per 64-part block
            xr12 = xr12p.tile([128, T12, NCHUNK], f16, tag="xr12")
            for a in range(2):
                src = bass.AP(tensor=xT[:].tensor, offset=a * XS + c * NCHUNK,
                              ap=[[0, 64], [2 * XS, T12], [1, NCHUNK]])
                nc.sync.dma_start(out=xr12[64 * a:64 * (a + 1), :, :], in_=src)
            st[c]["xr12"] = xr12

        def dma_xr0(c):
            # xr0[p, t, j] = x[4t + p//32, n0+j]; one DMA per 32-part block
            xr0 = xr0p.tile([128, T0, NCHUNK], f16, tag="xr0")
            for a in range(4):
                src = bass.AP(tensor=xT[:].tensor, offset=a * XS + c * NCHUNK,
                              ap=[[0, 32], [4 * XS, T0], [1, NCHUNK]])
                nc.scalar.dma_start(out=xr0[32 * a:32 * (a + 1), :, :], in_=src)
            st[c]["xr0"] = xr0

        def z0_mult(c):
            # z0 = xr0 * x4-bcast, in place (h-side g = p%32)
            xr0 = st[c]["xr0"]
            x4b = bass.AP(tensor=x4.tensor, offset=x4.offset + c * NCHUNK,
                          ap=[x4.ap[0], [0, T0], [1, NCHUNK]])
            nc.vector.tensor_mul(xr0[:], xr0[:], x4b)

        def hbcast(h, ntiles):
            return bass.AP(tensor=h.tensor, offset=h.offset,
                           ap=[h.ap[0], [0, ntiles], [1, NCHUNK]])

        def z1_mult_a(c):
            # z1 = xr12 * [h1;h1]-bcast, out of place (xr12 reused by z2).
            # First DVE half + the Pool share; second DVE half is emitted
            # later (z1_mult_b) so h2copy/z2 don't queue behind all of z1.
            xr12 = st[c]["xr12"]
            h1 = st[c]["h1"]
            z1 = z1p.tile([128, T12, NCHUNK], f16, tag="z1")
            nd = T12 - POOL_Z1
            na = nd // 2
            nc.vector.tensor_mul(z1[:, 0:na, :], xr12[:, 0:na, :],
                                 hbcast(h1, na))
            if POOL_Z1:
                hb = bass.AP(tensor=h1.tensor, offset=h1.offset,
                             ap=[h1.ap[0], [0, POOL_Z1], [1, NCHUNK]])
                nc.gpsimd.tensor_mul(z1[:, nd:, :], xr12[:, nd:, :], hb)
            st[c]["z1"] = z1

        def z1_mult_b(c):
            xr12 = st[c]["xr12"]
            h1 = st[c]["h1"]
            z1 = st[c]["z1"]
            nd = T12 - POOL_Z1
            na = nd // 2
            hb = bass.AP(tensor=h1.tensor, offset=h1.offset,
                         ap=[h1.ap[0], [0, nd - na], [1, NCHUNK]])
            nc.vector.tensor_mul(z1[:, na:nd, :], xr12[:, na:nd, :], hb)

        def z2_mult(c):
            # z2 = xr12 * [h2;h2]-bcast, in place
            xr12 = st[c]["xr12"]
            h2 = st[c]["h2"]
            nd = T12 - POOL_Z2
            nc.vector.tensor_mul(xr12[:, 0:nd, :], xr12[:, 0:nd, :],
                                 hbcast(h2, nd))
            if POOL_Z2:
                hb = bass.AP(tensor=h2.tensor, offset=h2.offset,
                             ap=[h2.ap[0], [0, POOL_Z2], [1, NCHUNK]])
                nc.gpsimd.tensor_mul(xr12[:, nd:, :], xr12[:, nd:, :], hb)

        def mm(c, i):
            if i == 0:
                zt, T = st[c]["xr0"], T0
            elif i == 1:
                zt, T = st[c]["z1"], T12
            else:
                zt, T = st[c]["xr12"], T12
            psum = ps.tile([128, NCHUNK], f32, tag="ps")
            for t in range(T):
                nc.tensor.matmul(psum[:], wts[i][:, t, :], zt[:, t, :],
                                 start=(t == 0), stop=(t == T - 1))
            st[c][f"psum{i}"] = psum

        def evac0(c):
            # L0 keep -> kt01[0:64]; h1 -> rows 64:128 then dup down
            psum = st[c]["psum0"]
            kt01 = kp.tile([128, NCHUNK], f32, tag="k01")
            nc.scalar.activation(out=kt01[0:64, :], in_=psum[0:64, :],
                                 func=Relu, bias=bt[0:64, 0:1], scale=1.0)
            hbuf = hp.tile([128, NCHUNK], f16, tag="h1")
            nc.scalar.activation(out=hbuf[64:128, :], in_=psum[64:128, :],
                                 func=Relu, bias=bt[64:128, 0:1], scale=1.0)
            nc.vector.tensor_copy(hbuf[0:64, :], hbuf[64:128, :])
            st[c]["k01"] = kt01
            st[c]["h1"] = hbuf

        def evac1(c):
            # W1 half-swapped: keep in psum rows 64:128, h2 in rows 0:64
            psum = st[c]["psum1"]
            kt01 = st[c]["k01"]
            nc.scalar.activation(out=kt01[64:128, :], in_=psum[64:128, :],
                                 func=Relu, bias=bt[64:128, 1:2], scale=1.0)
            hbuf = hp.tile([128, NCHUNK], f16, tag="h2")
            nc.scalar.activation(out=hbuf[0:64, :], in_=psum[0:64, :],
                                 func=Relu, bias=bt[0:64, 1:2], scale=1.0)
            nc.vector.tensor_copy(hbuf[64:128, :], hbuf[0:64, :])
            st[c]["h2"] = hbuf

        def evac2(c):
            psum = st[c]["psum2"]
            kt = kp.tile([128, NCHUNK], f32, tag="k2")
            nc.scalar.activation(out=kt[:], in_=psum[:], func=Relu,
                                 bias=bt[:, 2:3], scale=1.0)
            st[c]["k2"] = kt

        def etree(c, key):
            # e-sum as a pairwise-add tree on Pool (gpsimd has no free-axis
            # tensor_reduce); 16 -> 8 -> 4 -> 2 -> 1 per sample
            kt = st[c][key]
            dst = (outA if key == "k01" else outB)
            v = kt[:].rearrange("p (b e) -> p b e", e=E)
            t1 = rp.tile([128, BCH, 8], f32, tag=f"t1{key}")
            nc.gpsimd.tensor_add(t1[:], v[:, :, 0:8], v[:, :, 8:16])
            t2 = rp.tile([128, BCH, 4], f32, tag=f"t2{key}")
            nc.gpsimd.tensor_add(t2[:], t1[:, :, 0:4], t1[:, :, 4:8])
            t3 = rp.tile([128, BCH, 2], f32, tag=f"t3{key}")
            nc.gpsimd.tensor_add(t3[:], t2[:, :, 0:2], t2[:, :, 2:4])
            do = bass.AP(tensor=dst.tensor, offset=dst.offset + c * BCH,
                         ap=[dst.ap[0], [1, BCH], [1, 1]])
            nc.gpsimd.tensor_add(do, t3[:, :, 0:1], t3[:, :, 1:2])

        # ---- skewed pipeline ----
        dma_xr0(0)
        dma_xr12(0)
        dma_xr0(1)
        dma_xr12(1)
        z0_mult(0)

        for s in range(NCHUNKS + 2):
            if s + 2 < NCHUNKS:
                dma_xr0(s + 2)
                dma_xr12(s + 2)
            if s < NCHUNKS:
                mm(s, 0)
            if 0 <= s - 1 < NCHUNKS:
                mm(s - 1, 1)
            if s - 2 >= 0:
                mm(s - 2, 2)
            if s + 1 < NCHUNKS:
                z0_mult(s + 1)
            if s < NCHUNKS:
                evac0(s)
                z1_mult_a(s)
            if 0 <= s - 1 < NCHUNKS:
                evac1(s - 1)
                z2_mult(s - 1)
            if s < NCHUNKS:
                z1_mult_b(s)
            if 0 <= s - 1 < NCHUNKS:
                etree(s - 1, "k01")
            if s - 2 >= 0:
                evac2(s - 2)
                etree(s - 2, "k2")

        nc.sync.dma_start(
            out=bass.AP(tensor=out[:].tensor, offset=0, ap=[[BC, 128], [1, BC]]),
            in_=outA[:])
        nc.sync.dma_start(
            out=bass.AP(tensor=out[:].tensor, offset=128 * BC,
                        ap=[[BC, 128], [1, BC]]),
            in_=outB[:])

    nc.compile()
    return nc


def _get_nc():
    if "nc" not in _CACHE:
        _CACHE["nc"] = _build_module()
    return _CACHE["nc"]


def _prep_inputs(x, W0, b0, W1, b1, W2, b2):
    """Host-side prep: shard batch, transpose/convert. Returns in_maps."""
    x = np.asarray(x, dtype=np.float32)
    Ws = [np.asarray(W, dtype=np.float32) for W in (W0, W1, W2)]
    bs = [np.asarray(b, dtype=np.float32) for b in (b0, b1, b2)]

    # half-swap W1/b1 output rows: keep -> psum rows 64:128, h2 -> rows 0:64
    perm = np.concatenate([np.arange(64, 128), np.arange(64)])
    Ws[1] = Ws[1][perm]
    bs[1] = bs[1][perm]
    wts = [np.ascontiguousarray(W.T).astype(np.float16) for W in Ws]
    bias = np.stack(bs, axis=1).astype(np.float32)  # (128, 3)
    p = np.arange(128)

    in_maps = []
    for core in range(N_CORES):
        xc = x[core * BC:(core + 1) * BC]  # (512, 32, 16)
        xTc = np.ascontiguousarray(
            xc.transpose(1, 0, 2).reshape(F, NTOT)).astype(np.float16)
        x4c = np.ascontiguousarray(xTc[p % 32])
        in_maps.append({
            "xT": xTc,
            "x4d": x4c,
            "w0": wts[0], "w1": wts[1], "w2": wts[2],
            "bias": bias,
        })
    return in_maps


def kernel(x, W0, b0, W1, b1, W2, b2, _trace=False):
    from concourse.bass_utils import run_bass_kernel_spmd

    nc = _get_nc()
    in_maps = _prep_inputs(x, W0, b0, W1, b1, W2, b2)
    res = run_bass_kernel_spmd(nc, in_maps, core_ids=list(range(N_CORES)),
                               trace=_trace)
    outs = [res.results[i]["out"] for i in range(N_CORES)]  # each (256, 512)
    full = np.concatenate(outs, axis=1)                     # (256, 4096)
    result = np.ascontiguousarray(full.T).astype(np.float32)
    if _trace:
        return result, res
    return result


# revision 16
# speedup vs baseline: 1.8129x; 1.0373x over previous
"""Trainium2 Bass kernel for CompressedInteractionNetwork (CIN) forward.

Reference computation (per sample):
  x0 = x (F=32 fields, E=16 embed), h = x
  layer i: z = outer(x0, h) over fields -> (F*G_i, E); y = relu(W_i @ z + b_i)
  layers 0,1: keep = y[:64] -> output, h = y[64:]
  layer 2: keep = y
  out = concat(keeps) summed over E -> (B, 256)

Strategy: pure data parallelism over batch (4096 -> 512 per core, 8 cores).
Per core, n = (b_local, e) is the matmul free dim (8192 total, chunks of 512).
Compute in fp16, fp32 PSUM / bias / relu / e-sum.

v2 (chunk-major, software-pipelined):
  - z tiles are c-major (c = f*G + g on partitions, tiled by 128):
      z[c, n] = Xrep[c, n] * Hrep[c, n]
  - Xrep for layers 1 and 2 is IDENTICAL (pure x replication): DMA'd once
    per chunk as a single 4D-AP transfer (2 MB), reused by both layers
    (z1 out-of-place, z2 in-place) -> 48 MB replication DMA instead of 80.
  - One DMA instruction per (chunk, layer-group) instead of one per K-tile:
    HWDGE issue count drops ~13x. No DMA and no multiplies on GPSIMD's
    SWDGE path (the old bottleneck: Pool.ENGINE was 95% busy).
  - Elementwise z-mult split DVE (2x fp16 mode) / Pool (tunable tile count);
    evacuations on Act; e-sum reduces on Pool; h-dup copy on DVE.
  - Skewed 3-stage pipeline (chunk c layer 0 || c-1 layer 1 || c-2 layer 2)
    keeps PE dense; outputs accumulate in SBUF, 2 final DMAs.
"""
import os
import numpy as np

B = 4096
F = 32
E = 16
O = 128
N_CORES = 8
BC = B // N_CORES          # 512 samples per core
NTOT = BC * E              # 8192 n-columns per core
NCHUNK = 512               # matmul free dim per chunk
NCHUNKS = NTOT // NCHUNK   # 16
BCH = NCHUNK // E          # 32 samples per chunk
T0 = 8                     # K-tiles layer 0 (C=1024)
T12 = 16                   # K-tiles layers 1,2 (C=2048)

# tunables
POOL_Z2 = int(os.environ.get("POOL_Z2", "2"))   # z2 K-tiles multiplied on Pool
POOL_Z1 = int(os.environ.get("POOL_Z1", "4"))   # z1 K-tiles multiplied on Pool
# W1/b1 output rows are half-swapped on the host so the L1 keep lands in
# psum rows 64:128 (aligned with out rows 64:128) and h2 in rows 0:64 —
# every Act evacuation is then partition-aligned.

_CACHE = {}


def _build_module():
    import concourse.bass as bass
    import concourse.bacc as bacc
    import concourse.tile as tile
    from concourse import mybir
    from contextlib import ExitStack

    f16 = mybir.dt.float16
    f32 = mybir.dt.float32
    Relu = mybir.ActivationFunctionType.Relu
    Alu = mybir.AluOpType
    AxX = mybir.AxisListType.X

    nc = bacc.Bacc(None, target_bir_lowering=False)

    xT = nc.dram_tensor("xT", [F, NTOT], f16, kind="ExternalInput")
    x4d = nc.dram_tensor("x4d", [128, NTOT], f16, kind="ExternalInput")
    wdr = [
        nc.dram_tensor("w0", [T0 * 128, O], f16, kind="ExternalInput"),
        nc.dram_tensor("w1", [T12 * 128, O], f16, kind="ExternalInput"),
        nc.dram_tensor("w2", [T12 * 128, O], f16, kind="ExternalInput"),
    ]
    bias = nc.dram_tensor("bias", [O, 3], f32, kind="ExternalInput")
    out = nc.dram_tensor("out", [2 * O, BC], f32, kind="ExternalOutput")

    XS = NTOT  # xT row stride (elements)

    with tile.TileContext(nc) as tc, ExitStack() as ctx:
        singles = ctx.enter_context(tc.tile_pool(name="singles", bufs=1))
        xr0p = ctx.enter_context(tc.tile_pool(name="xr0p", bufs=4))
        xr12p = ctx.enter_context(tc.tile_pool(name="xr12p", bufs=4))
        z1p = ctx.enter_context(tc.tile_pool(name="z1p", bufs=2))
        hp = ctx.enter_context(tc.tile_pool(name="hp", bufs=3))
        kp = ctx.enter_context(tc.tile_pool(name="kp", bufs=3))
        rp = ctx.enter_context(tc.tile_pool(name="rp", bufs=2))
        ps = ctx.enter_context(tc.tile_pool(name="ps", bufs=6, space="PSUM"))

        # ---- persistent tiles + preloads ----
        x4 = singles.tile([128, NTOT], f16)       # x4[p] = x[p%32] (L0 h-side)
        # preload x4 in quarters so the first chunks' columns land early
        QP = NTOT // 4
        for q in range(4):
            nc.scalar.dma_start(
                out=x4[:, q * QP:(q + 1) * QP],
                in_=bass.AP(tensor=x4d[:].tensor, offset=q * QP,
                            ap=[[NTOT, 128], [1, QP]]))
        wts = []
        for i, T in enumerate((T0, T12, T12)):
            wt = singles.tile([128, T, O], f16, tag=f"w{i}")
            src = bass.AP(tensor=wdr[i][:].tensor, offset=0,
                          ap=[[O, 128], [128 * O, T], [1, O]])
            nc.sync.dma_start(out=wt[:], in_=src)
            wts.append(wt)
        bt = singles.tile([O, 3], f32)
        nc.sync.dma_start(out=bt[:], in_=bias[:])
        outA = singles.tile([128, BC], f32)   # rows 0:64 L0-keep, 64:128 L1-keep
        outB = singles.tile([128, BC], f32)   # L2 keep

        st = [dict() for _ in range(NCHUNKS)]

        def dma_xr12(c):
            # xr12[p, t, j] = x[2t + p//64, n0+j]; one DMA per 64-part block
            xr12 = xr12p.tile([128, T12, NCHUNK], f16, tag="xr12")
            for a in range(2):
                src = bass.AP(tensor=xT[:].tensor, offset=a * XS + c * NCHUNK,
                              ap=[[0, 64], [2 * XS, T12], [1, NCHUNK]])
                nc.sync.dma_start(out=xr12[64 * a:64 * (a + 1), :, :], in_=src)
            st[c]["xr12"] = xr12

        def dma_xr0(c):
            # xr0[p, t, j] = x[4t + p//32, n0+j]; one DMA per 32-part block
            xr0 = xr0p.tile([128, T0, NCHUNK], f16, tag="xr0")
            for a in range(4):
                src = bass.AP(tensor=xT[:].tensor, offset=a * XS + c * NCHUNK,
                              ap=[[0, 32], [4 * XS, T0], [1, NCHUNK]])
                nc.scalar.dma_start(out=xr0[32 * a:32 * (a + 1), :, :], in_=src)
            st[c]["xr0"] = xr0

        def z0_mult(c):
            # z0 = xr0 * x4-bcast, in place (h-side g = p%32)
            xr0 = st[c]["xr0"]
            x4b = bass.AP(tensor=x4.tensor, offset=x4.offset + c * NCHUNK,
                          ap=[x4.ap[0], [0, T0], [1, NCHUNK]])
            nc.vector.tensor_mul(xr0[:], xr0[:], x4b)

        def hbcast(h, ntiles):
            return bass.AP(tensor=h.tensor, offset=h.offset,
                           ap=[h.ap[0], [0, ntiles], [1, NCHUNK]])

        def z1_mult_a(c):
            # z1 = xr12 * [h1;h1]-bcast, out of place (xr12 reused by z2).
            # First DVE half + the Pool share; second DVE half is emitted
            # later (z1_mult_b) so h2copy/z2 don't queue behind all of z1.
            xr12 = st[c]["xr12"]
            h1 = st[c]["h1"]
            z1 = z1p.tile([128, T12, NCHUNK], f16, tag="z1")
            nd = T12 - POOL_Z1
            na = nd // 2
            nc.vector.tensor_mul(z1[:, 0:na, :], xr12[:, 0:na, :],
                                 hbcast(h1, na))
            if POOL_Z1:
                hb = bass.AP(tensor=h1.tensor, offset=h1.offset,
                             ap=[h1.ap[0], [0, POOL_Z1], [1, NCHUNK]])
                nc.gpsimd.tensor_mul(z1[:, nd:, :], xr12[:, nd:, :], hb)
            st[c]["z1"] = z1

        def z1_mult_b(c):
            xr12 = st[c]["xr12"]
            h1 = st[c]["h1"]
            z1 = st[c]["z1"]
            nd = T12 - POOL_Z1
            na = nd // 2
            hb = bass.AP(tensor=h1.tensor, offset=h1.offset,
                         ap=[h1.ap[0], [0, nd - na], [1, NCHUNK]])
            nc.vector.tensor_mul(z1[:, na:nd, :], xr12[:, na:nd, :], hb)

        def z2_mult(c):
            # z2 = xr12 * [h2;h2]-bcast, in place
            xr12 = st[c]["xr12"]
            h2 = st[c]["h2"]
            nd = T12 - POOL_Z2
            nc.vector.tensor_mul(xr12[:, 0:nd, :], xr12[:, 0:nd, :],
                                 hbcast(h2, nd))
            if POOL_Z2:
                hb = bass.AP(tensor=h2.tensor, offset=h2.offset,
                             ap=[h2.ap[0], [0, POOL_Z2], [1, NCHUNK]])
                nc.gpsimd.tensor_mul(xr12[:, nd:, :], xr12[:, nd:, :], hb)

        def mm(c, i):
            if i == 0:
                zt, T = st[c]["xr0"], T0
            elif i == 1:
                zt, T = st[c]["z1"], T12
            else:
                zt, T = st[c]["xr12"], T12
            psum = ps.tile([128, NCHUNK], f32, tag="ps")
            for t in range(T):
                nc.tensor.matmul(psum[:], wts[i][:, t, :], zt[:, t, :],
                                 start=(t == 0), stop=(t == T - 1))
            st[c][f"psum{i}"] = psum

        def evac0(c):
            # L0 keep -> kt01[0:64]; h1 -> rows 64:128 then dup down
            psum = st[c]["psum0"]
            kt01 = kp.tile([128, NCHUNK], f32, tag="k01")
            nc.scalar.activation(out=kt01[0:64, :], in_=psum[0:64, :],
                                 func=Relu, bias=bt[0:64, 0:1], scale=1.0)
            hbuf = hp.tile([128, NCHUNK], f16, tag="h1")
            nc.scalar.activation(out=hbuf[64:128, :], in_=psum[64:128, :],
                                 func=Relu, bias=bt[64:128, 0:1], scale=1.0)
            nc.vector.tensor_copy(hbuf[0:64, :], hbuf[64:128, :])
            st[c]["k01"] = kt01
            st[c]["h1"] = hbuf

        def evac1(c):
            # W1 half-swapped: keep in psum rows 64:128, h2 in rows 0:64
            psum = st[c]["psum1"]
            kt01 = st[c]["k01"]
            nc.scalar.activation(out=kt01[64:128, :], in_=psum[64:128, :],
                                 func=Relu, bias=bt[64:128, 1:2], scale=1.0)
            hbuf = hp.tile([128, NCHUNK], f16, tag="h2")
            nc.scalar.activation(out=hbuf[0:64, :], in_=psum[0:64, :],
                                 func=Relu, bias=bt[0:64, 1:2], scale=1.0)
            nc.vector.tensor_copy(hbuf[64:128, :], hbuf[0:64, :])
            st[c]["h2"] = hbuf

        def evac2(c):
            psum = st[c]["psum2"]
            kt = kp.tile([128, NCHUNK], f32, tag="k2")
            nc.scalar.activation(out=kt[:], in_=psum[:], func=Relu,
                                 bias=bt[:, 2:3], scale=1.0)
            st[c]["k2"] = kt

        def etree(c, key):
            # e-sum as a pairwise-add tree on Pool (gpsimd has no free-axis
            # tensor_reduce); 16 -> 8 -> 4 -> 2 -> 1 per sample
            kt = st[c][key]
            dst = (outA if key == "k01" else outB)
            v = kt[:].rearrange("p (b e) -> p b e", e=E)
            t1 = rp.tile([128, BCH, 8], f32, tag=f"t1{key}")
            nc.gpsimd.tensor_add(t1[:], v[:, :, 0:8], v[:, :, 8:16])
            t2 = rp.tile([128, BCH, 4], f32, tag=f"t2{key}")
            nc.gpsimd.tensor_add(t2[:], t1[:, :, 0:4], t1[:, :, 4:8])
            t3 = rp.tile([128, BCH, 2], f32, tag=f"t3{key}")
            nc.gpsimd.tensor_add(t3[:], t2[:, :, 0:2], t2[:, :, 2:4])
            do = bass.AP(tensor=dst.tensor, offset=dst.offset + c * BCH,
                         ap=[dst.ap[0], [1, BCH], [1, 1]])
            nc.gpsimd.tensor_add(do, t3[:, :, 0:1], t3[:, :, 1:2])

        # ---- skewed pipeline ----
        dma_xr0(0)
        dma_xr12(0)
        dma_xr0(1)
        dma_xr12(1)
        z0_mult(0)

        for s in range(NCHUNKS + 2):
            if s + 2 < NCHUNKS:
                dma_xr12(s + 2)
            if s < NCHUNKS:
                mm(s, 0)
            if 0 <= s - 1 < NCHUNKS:
                mm(s - 1, 1)
            if s - 2 >= 0:
                mm(s - 2, 2)
            if s + 1 < NCHUNKS:
                z0_mult(s + 1)
            if s < NCHUNKS:
                evac0(s)
                z1_mult_a(s)
            if 0 <= s - 1 < NCHUNKS:
                evac1(s - 1)
                z2_mult(s - 1)
            if s < NCHUNKS:
                z1_mult_b(s)
            if 0 <= s - 1 < NCHUNKS:
                etree(s - 1, "k01")
            if s - 2 >= 0:
                evac2(s - 2)
                etree(s - 2, "k2")
            if s + 2 < NCHUNKS:
                dma_xr0(s + 2)

        nc.sync.dma_start(
            out=bass.AP(tensor=out[:].tensor, offset=0, ap=[[BC, 128], [1, BC]]),
            in_=outA[:])
        nc.sync.dma_start(
            out=bass.AP(tensor=out[:].tensor, offset=128 * BC,
                        ap=[[BC, 128], [1, BC]]),
            in_=outB[:])

    nc.compile()
    return nc


def _get_nc():
    if "nc" not in _CACHE:
        _CACHE["nc"] = _build_module()
    return _CACHE["nc"]


def _prep_inputs(x, W0, b0, W1, b1, W2, b2):
    """Host-side prep: shard batch, transpose/convert. Returns in_maps."""
    x = np.asarray(x, dtype=np.float32)
    Ws = [np.asarray(W, dtype=np.float32) for W in (W0, W1, W2)]
    bs = [np.asarray(b, dtype=np.float32) for b in (b0, b1, b2)]

    # half-swap W1/b1 output rows: keep -> psum rows 64:128, h2 -> rows 0:64
    perm = np.concatenate([np.arange(64, 128), np.arange(64)])
    Ws[1] = Ws[1][perm]
    bs[1] = bs[1][perm]
    wts = [np.ascontiguousarray(W.T).astype(np.float16) for W in Ws]
    bias = np.stack(bs, axis=1).astype(np.float32)  # (128, 3)
    p = np.arange(128)

    in_maps = []
    for core in range(N_CORES):
        xc = x[core * BC:(core + 1) * BC]  # (512, 32, 16)
        xTc = np.ascontiguousarray(
            xc.transpose(1, 0, 2).reshape(F, NTOT)).astype(np.float16)
        x4c = np.ascontiguousarray(xTc[p % 32])
        in_maps.append({
            "xT": xTc,
            "x4d": x4c,
            "w0": wts[0], "w1": wts[1], "w2": wts[2],
            "bias": bias,
        })
    return in_maps


def kernel(x, W0, b0, W1, b1, W2, b2, _trace=False):
    from concourse.bass_utils import run_bass_kernel_spmd

    nc = _get_nc()
    in_maps = _prep_inputs(x, W0, b0, W1, b1, W2, b2)
    res = run_bass_kernel_spmd(nc, in_maps, core_ids=list(range(N_CORES)),
                               trace=_trace)
    outs = [res.results[i]["out"] for i in range(N_CORES)]  # each (256, 512)
    full = np.concatenate(outs, axis=1)                     # (256, 4096)
    result = np.ascontiguousarray(full.T).astype(np.float32)
    if _trace:
        return result, res
    return result
